# revision 7
# baseline (speedup 1.0000x reference)
"""Trainium2 Bass kernel: single-channel Conv2d.

  x: [32, 224, 224] f32, kernels: [64, 7, 7] f32
  out[b, k, i, j] = sum_{di,dj} x[b, i+di, j+dj] * kernels[k, di, dj]
  -> [32, 64, 218, 218]

Sharding: data-parallel over batch, 4 images per NeuronCore across 8 cores.

Per-core algorithm (bf16 matmuls, one stationary weight per PE half):
  - Host sends x as bf16 pre-interleaved per image-pair
    (xh[qp, row, img*224+j]) and a banded stationary matrix
        W[dr*8 + g, s*64 + k] = kernels[k, dr - s, g]   (dr 0..7, g 0..7,
    s 0..1; zero outside 0 <= dr-s <= 6, g <= 6) duplicated at PE rows
    0..63 and 64..127.  All 49 taps live in one 64-deep contraction, so
    every output-row-pair needs exactly ONE matmul.
  - An image-pair's rows are staged in SBUF as x2b[row, seg*464 + u]
    (u = img*224 + j; segments rows 0..127 / 96..223; 16-col zero pad).
  - ONE gather DMA builds pt[p = dr*8+g, u] = x2b[r0 + dr, seg_off+u+g]
    for TWO row-pairs at once (dr 0..15: rows r0..r0+15 feed pairs r0/2
    and r0/2+4): the 8 column shifts are overlapping stride-1 dims of the
    source AP, so no shift-expanded image copy is ever materialized.
  - Per row-pair, one matmul into ps[128 = (s,k), 448 = (img,j)]:
    pair A uses PE rows 0..63 (rhs/lhsT base 0), pair B PE rows 64..127.
  - VectorE+ScalarE evacuate PSUM into a 16-pair SBUF chunk [128, 16*448].
  - Each chunk is stored VERBATIM to DRAM (one DMA, 128 x 28.7KB fully
    contiguous descriptors) on the Pool (SWDGE) queue; the host undoes the
    (q, chunk, (s,k), (pl,img,j)) layout with a single numpy transpose.
    This keeps the SDMA engines byte-bound instead of descriptor-bound.
"""
import sys

sys.path.insert(0, "/opt/trn_rl_repo")

import numpy as np
import ml_dtypes

B, H, W = 32, 224, 224
KCH, KS = 64, 7
HO = WO = H - KS + 1  # 218
NCORES = 8
BLOC = B // NCORES    # 4 images per core
NPAIRS = HO // 2      # 109 output-row-pairs per image-pair

SEGW = 464            # x2b per-segment span (448 data + 16 zero pad)
X2F = 2 * SEGW        # 928
SEG1 = 96             # first row of segment 1 (rows 96..223)
NST = 448             # matmul stream length (2 imgs x 224)
DVE_COLS = 268        # PSUM evacuation split: VectorE cols, rest ScalarE
CH = 16               # row-pairs per output SBUF chunk
NCHUNK = 7            # chunks per image-pair (6*16 + 13 = 109)

_NC_CACHE = {}


def make_weight_band(kernels: np.ndarray) -> np.ndarray:
    """Stationary matrix [128, 128] (bf16): the 64-row band
    W[dr*8 + g, s*64 + k] = kernels[k, dr - s, g], duplicated at
    partitions 0..63 and 64..127 (PE quadrant rows 0 / 64)."""
    wb = np.zeros((64, 128), dtype=np.float32)
    for dr in range(8):
        for g in range(KS):
            for s in range(2):
                di = dr - s
                if 0 <= di < KS:
                    wb[dr * 8 + g, s * KCH: (s + 1) * KCH] = kernels[:, di, g]
    return np.vstack([wb, wb]).astype(ml_dtypes.bfloat16)


def _build_nc():
    import concourse.bacc as bacc
    import concourse.mybir as mybir
    import concourse.tile as tile
    from concourse.bass_types import AP

    F32 = mybir.dt.float32
    BF16 = mybir.dt.bfloat16

    nc = bacc.Bacc("TRN2", target_bir_lowering=False, debug=False,
                   num_devices=NCORES)
    # x pre-interleaved on host: [image-pair, row, img*224+j]
    x_d = nc.dram_tensor("x", [2, H, 2 * W], BF16, kind="ExternalInput").ap()
    wb_d = nc.dram_tensor("wband", [128, 128], BF16,
                          kind="ExternalInput").ap()
    # raw chunk dump: host untangles the layout
    out_d = nc.dram_tensor("out", [2, NCHUNK, 128, CH * NST], F32,
                           kind="ExternalOutput").ap()

    with tile.TileContext(nc) as tc:
        with (
            tc.tile_pool(name="wpool", bufs=1) as wpool,
            tc.tile_pool(name="x2pool", bufs=2) as x2pool,
            tc.tile_pool(name="ptpool", bufs=6) as ptpool,
            tc.tile_pool(name="opool", bufs=3) as opool,
            tc.tile_pool(name="psum", bufs=4, space="PSUM") as psum,
        ):
            wbt = wpool.tile([128, 128], BF16)
            nc.sync.dma_start(out=wbt[:], in_=wb_d)

            for q in range(2):
                x2b = x2pool.tile([128, X2F], BF16, tag="x2b")
                # zero the 16-col pads (cols 448..463 / 912..927)
                nc.gpsimd.memset(x2b[:, 448:464], 0.0)
                nc.gpsimd.memset(x2b[:, 912:928], 0.0)
                for seg in range(2):
                    r_lo = 0 if seg == 0 else SEG1
                    nc.sync.dma_start(
                        out=x2b[0:128, seg * SEGW: seg * SEGW + 2 * W],
                        in_=x_d[q, r_lo: r_lo + 128, :],
                    )

                chunks = {}  # chunk_start -> [tile, npl, n_evacuated]

                def get_chunk(pr):
                    cs = (pr // CH) * CH
                    if cs not in chunks:
                        npl = min(CH, NPAIRS - cs)
                        chunks[cs] = [opool.tile([128, CH * NST], F32,
                                                 tag="osb", name="chunk"),
                                      npl, 0]
                    return cs, chunks[cs]

                def evac_and_store(pr, ps, q):
                    cs, ent = get_chunk(pr)
                    chunk, npl = ent[0], ent[1]
                    pl = pr - cs
                    nc.vector.tensor_copy(
                        out=chunk[:, pl * NST: pl * NST + DVE_COLS],
                        in_=ps[:, 0:DVE_COLS])
                    nc.scalar.copy(
                        out=chunk[:, pl * NST + DVE_COLS: (pl + 1) * NST],
                        in_=ps[:, DVE_COLS:NST])
                    ent[2] += 1
                    if ent[2] == npl:
                        nc.gpsimd.dma_start(
                            out=out_d[q, cs // CH], in_=chunk[:])

                # groups of 8 pairs; 4 gathers per group, 2 pairs each
                for t in range(14):
                    for u in range(4):
                        prA = 8 * t + u
                        prB = prA + 4
                        if prA >= NPAIRS:
                            break
                        has_b = prB < NPAIRS
                        r0 = 2 * prA
                        nr = 16 if has_b else 8
                        seg = 0 if r0 + nr - 1 <= 127 else 1
                        roff = r0 - SEG1 * seg
                        pt = ptpool.tile([128, NST], BF16, tag="pt")
                        src = AP(
                            tensor=x2b[:].tensor,
                            offset=x2b[:].offset + roff * X2F + seg * SEGW,
                            ap=((X2F, nr), (1, 8), (1, NST)),
                        )
                        dst = AP(
                            tensor=pt[:].tensor,
                            offset=pt[:].offset,
                            ap=((NST, nr * 8), (1, NST)),
                        )
                        nc.sync.dma_start(out=dst, in_=src)

                        psA = psum.tile([128, NST], F32)
                        nc.tensor.matmul(
                            out=psA[:], lhsT=wbt[0:64, :],
                            rhs=pt[0:64, :], start=True, stop=True)
                        evac_and_store(prA, psA, q)
                        if has_b:
                            psB = psum.tile([128, NST], F32)
                            nc.tensor.matmul(
                                out=psB[:], lhsT=wbt[64:128, :],
                                rhs=pt[64:128, :], start=True, stop=True)
                            evac_and_store(prB, psB, q)
    nc.compile()
    return nc


def _get_nc():
    if "nc" not in _NC_CACHE:
        _NC_CACHE["nc"] = _build_nc()
    return _NC_CACHE["nc"]


def _prep_x(x: np.ndarray) -> np.ndarray:
    """[B, H, W] f32 -> per-core [2, H, 2W] bf16, images interleaved."""
    xb = x.astype(ml_dtypes.bfloat16)
    # [B,H,W] -> [B//2 pairs, 2, H, W] -> [pairs, H, 2, W] -> [pairs, H, 2W]
    xp = xb.reshape(B // 2, 2, H, W).transpose(0, 2, 1, 3).reshape(
        B // 2, H, 2 * W)
    return np.ascontiguousarray(xp)


def _untangle(arr: np.ndarray) -> np.ndarray:
    """Per-core raw chunk dump [2, NCHUNK, 128, CH*NST] f32 ->
    [4, KCH, HO, WO]."""
    # [q, ci, s*64+k, pl*448 + img*224 + j]
    a = arr.reshape(2, NCHUNK, 2, KCH, CH, 2, W)      # q ci s k pl img j
    a = a.transpose(0, 5, 3, 1, 4, 2, 6)              # q img k ci pl s j
    a = a.reshape(BLOC, KCH, 2 * NCHUNK * CH, W)      # rows = 224
    return a[:, :, :HO, :WO]


def kernel(x: np.ndarray, kernels: np.ndarray) -> np.ndarray:
    from concourse.bass_utils import run_bass_kernel_spmd

    x = np.asarray(x, dtype=np.float32)
    kernels = np.asarray(kernels, dtype=np.float32)
    xp = _prep_x(x)  # [16, H, 448]
    wb = make_weight_band(kernels)
    nc = _get_nc()
    in_maps = [
        {"x": xp[c * 2: c * 2 + 2], "wband": wb}
        for c in range(NCORES)
    ]
    res = run_bass_kernel_spmd(nc, in_maps, core_ids=list(range(NCORES)))
    return np.ascontiguousarray(np.concatenate(
        [_untangle(res.results[c]["out"]) for c in range(NCORES)], axis=0))


# revision 12
# speedup vs baseline: 79604.7644x; 79604.7644x over previous
"""Trainium2 Bass kernel: single-channel Conv2d.

  x: [32, 224, 224] f32, kernels: [64, 7, 7] f32
  out[b, k, i, j] = sum_{di,dj} x[b, i+di, j+dj] * kernels[k, di, dj]
  -> [32, 64, 218, 218]

Sharding: data-parallel over batch, 4 images per NeuronCore across 8 cores.

Per-core algorithm (bf16 matmuls, one stationary weight per PE half):
  - Host sends x as bf16 pre-interleaved per image-pair
    (xh[qp, row, img*224+j]) and a banded stationary matrix
        W[dr*8 + g, s*64 + k] = kernels[k, dr - s, g]   (dr 0..7, g 0..7,
    s 0..1; zero outside 0 <= dr-s <= 6, g <= 6) duplicated at PE rows
    0..63 and 64..127.  All 49 taps live in one 64-deep contraction, so
    every output-row-pair needs exactly ONE matmul.
  - An image-pair's rows are staged in SBUF as x2b[row, seg*464 + u]
    (u = img*224 + j; segments rows 0..127 / 96..223; 16-col zero pad).
  - ONE gather DMA builds pt[p = dr*8+g, u] = x2b[r0 + dr, seg_off+u+g]
    for TWO row-pairs at once (dr 0..15: rows r0..r0+15 feed pairs r0/2
    and r0/2+4): the 8 column shifts are overlapping stride-1 dims of the
    source AP, so no shift-expanded image copy is ever materialized.
  - Per row-pair, one matmul into ps[128 = (s,k), 448 = (img,j)]:
    pair A uses PE rows 0..63 (rhs/lhsT base 0), pair B PE rows 64..127.
  - VectorE+ScalarE evacuate PSUM into a 16-pair SBUF chunk [128, 16*448].
  - Each chunk is stored VERBATIM to DRAM (one DMA, 128 x 28.7KB fully
    contiguous descriptors) on the Pool (SWDGE) queue; the host undoes the
    (q, chunk, (s,k), (pl,img,j)) layout with a single numpy transpose.
    This keeps the SDMA engines byte-bound instead of descriptor-bound.
"""
import sys

sys.path.insert(0, "/opt/trn_rl_repo")

import numpy as np
import ml_dtypes

B, H, W = 32, 224, 224
KCH, KS = 64, 7
HO = WO = H - KS + 1  # 218
NCORES = 8
BLOC = B // NCORES    # 4 images per core
NPAIRS = HO // 2      # 109 output-row-pairs per image-pair

SEGW = 464            # x2b per-segment span (448 data + 16 zero pad)
X2F = 2 * SEGW        # 928
SEG1 = 96             # first row of segment 1 (rows 96..223)
NST = 448             # matmul stream length (2 imgs x 224)
DVE_COLS = 268        # PSUM evacuation split: VectorE cols, rest ScalarE
CH = 16               # row-pairs per output SBUF chunk
NCHUNK = 7            # chunks per image-pair (6*16 + 13 = 109)

_NC_CACHE = {}


def make_weight_band(kernels: np.ndarray) -> np.ndarray:
    """Stationary matrix [128, 128] (bf16): the 64-row band
    W[dr*8 + g, s*64 + k] = kernels[k, dr - s, g], duplicated at
    partitions 0..63 and 64..127 (PE quadrant rows 0 / 64)."""
    wb = np.zeros((64, 128), dtype=np.float32)
    for dr in range(8):
        for g in range(KS):
            for s in range(2):
                di = dr - s
                if 0 <= di < KS:
                    wb[dr * 8 + g, s * KCH: (s + 1) * KCH] = kernels[:, di, g]
    return np.vstack([wb, wb]).astype(ml_dtypes.bfloat16)


def _build_nc():
    import concourse.bacc as bacc
    import concourse.mybir as mybir
    import concourse.tile as tile
    from concourse.bass_types import AP

    F32 = mybir.dt.float32
    BF16 = mybir.dt.bfloat16

    nc = bacc.Bacc("TRN2", target_bir_lowering=False, debug=False,
                   num_devices=NCORES)
    # x pre-interleaved on host: [image-pair, row, img*224+j]
    x_d = nc.dram_tensor("x", [2, H, 2 * W], BF16, kind="ExternalInput").ap()
    wb_d = nc.dram_tensor("wband", [128, 128], BF16,
                          kind="ExternalInput").ap()
    # raw chunk dump (bf16; host untangles the layout and upcasts)
    out_d = nc.dram_tensor("out", [2, NCHUNK, 128, CH * NST], BF16,
                           kind="ExternalOutput").ap()

    with tile.TileContext(nc) as tc:
        with (
            tc.tile_pool(name="wpool", bufs=1) as wpool,
            tc.tile_pool(name="x2pool", bufs=2) as x2pool,
            tc.tile_pool(name="ptpool", bufs=6) as ptpool,
            tc.tile_pool(name="opool", bufs=3) as opool,
            tc.tile_pool(name="psum", bufs=4, space="PSUM") as psum,
        ):
            wbt = wpool.tile([128, 128], BF16)
            nc.sync.dma_start(out=wbt[:], in_=wb_d)

            for q in range(2):
                x2b = x2pool.tile([128, X2F], BF16, tag="x2b")
                # zero the 16-col pads (cols 448..463 / 912..927)
                nc.gpsimd.memset(x2b[:, 448:464], 0.0)
                nc.gpsimd.memset(x2b[:, 912:928], 0.0)
                for seg in range(2):
                    r_lo = 0 if seg == 0 else SEG1
                    nc.sync.dma_start(
                        out=x2b[0:128, seg * SEGW: seg * SEGW + 2 * W],
                        in_=x_d[q, r_lo: r_lo + 128, :],
                    )

                chunks = {}  # chunk_start -> [tile, npl, n_evacuated]

                def get_chunk(pr):
                    cs = (pr // CH) * CH
                    if cs not in chunks:
                        npl = min(CH, NPAIRS - cs)
                        chunks[cs] = [opool.tile([128, CH * NST], BF16,
                                                 tag="osb", name="chunk"),
                                      npl, 0]
                    return cs, chunks[cs]

                def evac_and_store(pr, ps, q):
                    cs, ent = get_chunk(pr)
                    chunk, npl = ent[0], ent[1]
                    pl = pr - cs
                    nc.vector.tensor_copy(
                        out=chunk[:, pl * NST: pl * NST + DVE_COLS],
                        in_=ps[:, 0:DVE_COLS])
                    nc.scalar.copy(
                        out=chunk[:, pl * NST + DVE_COLS: (pl + 1) * NST],
                        in_=ps[:, DVE_COLS:NST])
                    ent[2] += 1
                    if ent[2] == npl:
                        nc.gpsimd.dma_start(
                            out=out_d[q, cs // CH, :, 0: npl * NST],
                            in_=chunk[:, 0: npl * NST])

                # groups of 8 pairs; 4 gathers per group, 2 pairs each
                for t in range(14):
                    for u in range(4):
                        prA = 8 * t + u
                        prB = prA + 4
                        if prA >= NPAIRS:
                            break
                        has_b = prB < NPAIRS
                        r0 = 2 * prA
                        nr = 16 if has_b else 8
                        seg = 0 if r0 + nr - 1 <= 127 else 1
                        roff = r0 - SEG1 * seg
                        pt = ptpool.tile([128, NST], BF16, tag="pt")
                        src = AP(
                            tensor=x2b[:].tensor,
                            offset=x2b[:].offset + roff * X2F + seg * SEGW,
                            ap=((X2F, nr), (1, 8), (1, NST)),
                        )
                        dst = AP(
                            tensor=pt[:].tensor,
                            offset=pt[:].offset,
                            ap=((NST, nr * 8), (1, NST)),
                        )
                        g_eng = nc.sync if (t + u) % 2 == 0 else nc.gpsimd
                        g_eng.dma_start(out=dst, in_=src)

                        psA = psum.tile([128, NST], F32)
                        nc.tensor.matmul(
                            out=psA[:], lhsT=wbt[0:64, :],
                            rhs=pt[0:64, :], start=True, stop=True)
                        evac_and_store(prA, psA, q)
                        if has_b:
                            psB = psum.tile([128, NST], F32)
                            nc.tensor.matmul(
                                out=psB[:], lhsT=wbt[64:128, :],
                                rhs=pt[64:128, :], start=True, stop=True)
                            evac_and_store(prB, psB, q)
    nc.compile()
    return nc


def _get_nc():
    if "nc" not in _NC_CACHE:
        _NC_CACHE["nc"] = _build_nc()
    return _NC_CACHE["nc"]


def _prep_x(x: np.ndarray) -> np.ndarray:
    """[B, H, W] f32 -> per-core [2, H, 2W] bf16, images interleaved."""
    xb = x.astype(ml_dtypes.bfloat16)
    # [B,H,W] -> [B//2 pairs, 2, H, W] -> [pairs, H, 2, W] -> [pairs, H, 2W]
    xp = xb.reshape(B // 2, 2, H, W).transpose(0, 2, 1, 3).reshape(
        B // 2, H, 2 * W)
    return np.ascontiguousarray(xp)


def _untangle(arr: np.ndarray) -> np.ndarray:
    """Per-core raw chunk dump [2, NCHUNK, 128, CH*NST] bf16 ->
    [4, KCH, HO, WO] f32."""
    # [q, ci, s*64+k, pl*448 + img*224 + j]
    a = arr.reshape(2, NCHUNK, 2, KCH, CH, 2, W)      # q ci s k pl img j
    a = a.transpose(0, 5, 3, 1, 4, 2, 6)              # q img k ci pl s j
    a = a.reshape(BLOC, KCH, 2 * NCHUNK * CH, W)      # rows = 224
    return a[:, :, :HO, :WO].astype(np.float32)


def kernel(x: np.ndarray, kernels: np.ndarray) -> np.ndarray:
    from concourse.bass_utils import run_bass_kernel_spmd

    x = np.asarray(x, dtype=np.float32)
    kernels = np.asarray(kernels, dtype=np.float32)
    xp = _prep_x(x)  # [16, H, 448]
    wb = make_weight_band(kernels)
    nc = _get_nc()
    in_maps = [
        {"x": xp[c * 2: c * 2 + 2], "wband": wb}
        for c in range(NCORES)
    ]
    res = run_bass_kernel_spmd(nc, in_maps, core_ids=list(range(NCORES)))
    return np.ascontiguousarray(np.concatenate(
        [_untangle(res.results[c]["out"]) for c in range(NCORES)], axis=0))
